# revision 18
# baseline (speedup 1.0000x reference)
"""AntiSymmetricDGN on 8 TRN2 NeuronCores (Bass/Tile, SPMD).

Strategy (node-sharded graph parallel):
  - nodes sharded 8x6250; per-core node state kept TRANSPOSED in SBUF
    [feat(partitions), nodes(free)].
  - per GCN iteration: hw^T = gcn_w^T @ h^T locally, PE-transpose to
    node-major bf16 rows, AllGather into a replicated HBM table
    [50000, 128] bf16.
  - irregular gather: SWDGE dma_gather of hw rows by src index (two
    calls per superblock: table halves, since idx is int16).
  - scatter/segment-sum: TensorE matmuls. Edges are sorted by dst and
    packed into 128-edge chunks; chunk matmul: lhsT = gathered rows
    [128 edges, 128 feat] (stationary), rhs = S^T coefficient chunk
    [128 edges, 32 dst-cols] -> accumulates y^T [feat, dst] in PSUM.
    Chunk counts/capacities are made uniform across cores (SPMD: one
    instruction stream) by padding to the max over cores per fixed
    32-column dst window.
  - dense ops (x@w_hid.T, h@aW.T, tanh, leaky_relu, log_softmax) in
    f32 on PE/DVE/ACT in the transposed layout.
"""
import math
import os
import numpy as np

import concourse.bass as bass
from concourse import mybir, bacc
from concourse.bass_utils import run_bass_kernel_spmd
from concourse.tile import TileContext

# problem constants
N, E, IN, H, H2, OUT = 50000, 600000, 256, 128, 64, 40
EPS, GAMMA = 0.1, 0.1
NCORES = 8
NSH = N // NCORES          # 6250 nodes per core
WCOL = 32                  # dst window width (S^T chunk width)
NW = (NSH + WCOL - 1) // WCOL      # 196 windows per core (last=10 cols)
SBW = 16                   # windows per superblock (512 dst cols)
NSB = (NW + SBW - 1) // SBW        # 13 superblocks
HALF = N // 2              # table half split for int16 gather indices
CHUNK = 128

F32 = mybir.dt.float32
BF16 = mybir.dt.bfloat16
I16 = mybir.dt.int16
AF = mybir.ActivationFunctionType
ALU = mybir.AluOpType


# ----------------------------------------------------------------- prep
def _prep_graph(edge_index):
    src = np.asarray(edge_index[0], dtype=np.int64)
    dst = np.asarray(edge_index[1], dtype=np.int64)
    loops = np.arange(N, dtype=np.int64)
    srcL = np.concatenate([src, loops])
    dstL = np.concatenate([dst, loops])
    deg = np.bincount(dstL, minlength=N).astype(np.float32)
    dinv = (1.0 / np.sqrt(np.maximum(deg, 1e-12))).astype(np.float32)
    dinv[deg <= 0] = 0.0
    norm = (dinv[srcL] * dinv[dstL]).astype(np.float32)

    core = dstL // NSH
    col = dstL % NSH
    win = col // WCOL
    src_core = srcL // NSH
    src_r = srcL % NSH
    halfB = (src_r >= NSH // 2).astype(np.int64)
    # row index within half-table h: src_core*(NSH//2) + (src_r - h*(NSH//2))
    tabrow = src_core * (NSH // 2) + (src_r - halfB * (NSH // 2))

    key = ((core * NW + win) * 2 + halfB)
    order = np.lexsort((col, key))
    srcS, colS, normS, keyS = tabrow[order], col[order], norm[order], key[order]

    # counts per (core, win, half)
    cnt = np.bincount(keyS, minlength=NCORES * NW * 2).reshape(NCORES, NW, 2)
    # uniform capacities per window (max over cores)
    CA = np.maximum(np.ceil(cnt[:, :, 0] / CHUNK).max(axis=0), 0).astype(np.int64)
    CB = np.maximum(np.ceil(cnt[:, :, 1] / CHUNK).max(axis=0), 0).astype(np.int64)

    # superblock layout
    sb_windows = [list(range(s * SBW, min((s + 1) * SBW, NW))) for s in range(NSB)]
    CA_sb = [int(sum(CA[w] for w in ws)) for ws in sb_windows]
    CB_sb = [int(sum(CB[w] for w in ws)) for ws in sb_windows]
    C_sb = [a + b for a, b in zip(CA_sb, CB_sb)]
    choff = np.concatenate([[0], np.cumsum(C_sb)]).astype(np.int64)   # chunk offsets per sb
    TOTCH = int(choff[-1])
    LA = sum(CA_sb) * CHUNK     # total A-gather slots per core
    LB = sum(CB_sb) * CHUNK

    # block index of chunk (w local in sb, half, k) within its superblock
    blkA_off, blkB_off = [], []
    for s, ws in enumerate(sb_windows):
        offs, acc = {}, 0
        for w in ws:
            offs[w] = acc
            acc += int(CA[w])
        blkA_off.append(offs)
        offs, acc = {}, 0
        for w in ws:
            offs[w] = acc
            acc += int(CB[w])
        blkB_off.append(offs)

    # group start offsets in the sorted arrays
    grp_start = np.concatenate([[0], np.cumsum(cnt.reshape(-1))]).astype(np.int64)

    idxA = np.zeros((NCORES, max(LA, 16)), dtype=np.int16)
    idxB = np.zeros((NCORES, max(LB, 16)), dtype=np.int16)
    scoef = np.zeros((NCORES, TOTCH, CHUNK, WCOL), dtype=np.float32)

    for c in range(NCORES):
        offA = offB = 0
        for s, ws in enumerate(sb_windows):
            for half in (0, 1):
                for w in ws:
                    g = (c * NW + w) * 2 + half
                    st, n = grp_start[g], int(cnt[c, w, half])
                    e_src = srcS[st:st + n]
                    e_col = colS[st:st + n]
                    e_nrm = normS[st:st + n]
                    cap = int((CA if half == 0 else CB)[w])
                    for k in range(cap):
                        lo = k * CHUNK
                        sl = slice(lo, min(lo + CHUNK, n))
                        m = sl.stop - sl.start
                        if half == 0:
                            blk = blkA_off[s][w] + k
                            ch = int(choff[s]) + blk
                            if m > 0:
                                idxA[c, offA:offA + m] = e_src[sl]
                            offA += CHUNK
                        else:
                            blk = CA_sb[s] + blkB_off[s][w] + k
                            ch = int(choff[s]) + blk
                            if m > 0:
                                idxB[c, offB:offB + m] = e_src[sl]
                            offB += CHUNK
                        if m > 0:
                            rel = (e_col[sl] - w * WCOL).astype(np.int64)
                            scoef[c, ch, np.arange(m), rel] = e_nrm[sl]
        assert offA == LA and offB == LB

    def wrap_idx(flat):
        L = len(flat)
        assert L % 16 == 0
        w16 = flat.reshape(L // 16, 16).T.copy()     # [16, L/16]
        return np.tile(w16, (8, 1))                   # [128, L/16]

    meta = dict(CA=CA, CB=CB, sb_windows=sb_windows, CA_sb=CA_sb, CB_sb=CB_sb,
                C_sb=C_sb, choff=choff, TOTCH=TOTCH, LA=LA, LB=LB,
                blkA_off=blkA_off, blkB_off=blkB_off)
    percore = []
    for c in range(NCORES):
        percore.append(dict(
            idxA=wrap_idx(idxA[c]),
            idxB=wrap_idx(idxB[c]),
            scoef=np.ascontiguousarray(
                scoef[c].transpose(1, 0, 2)
            ).astype(mybir.dt.np(BF16)),
        ))
    return meta, percore


# ---------------------------------------------------------------- build
def _build(meta):
    CA, CB = meta["CA"], meta["CB"]
    sb_windows = meta["sb_windows"]
    CA_sb, CB_sb, C_sb = meta["CA_sb"], meta["CB_sb"], meta["C_sb"]
    choff, TOTCH, LA, LB = meta["choff"], meta["TOTCH"], meta["LA"], meta["LB"]
    blkA_off, blkB_off = meta["blkA_off"], meta["blkB_off"]
    CSB_MAX = max(C_sb)

    nc = bacc.Bacc(num_devices=NCORES, num_swdge_queues=4)
    p_xT = nc.declare_dram_parameter("xT", [IN, NSH], F32, isOutput=False)
    p_idxA = nc.declare_dram_parameter("idxA", [128, LA // 16], I16, isOutput=False)
    p_idxB = nc.declare_dram_parameter("idxB", [128, LB // 16], I16, isOutput=False)
    p_sc = nc.declare_dram_parameter("scoef", [128, TOTCH, WCOL], BF16, isOutput=False)
    p_w0T = nc.declare_dram_parameter("w0T", [IN, H], F32, isOutput=False)
    p_b0 = nc.declare_dram_parameter("b0", [H, 1], F32, isOutput=False)
    p_aW1T = nc.declare_dram_parameter("aW1T", [H, H], F32, isOutput=False)
    p_gw1 = nc.declare_dram_parameter("gw1", [H, H], F32, isOutput=False)
    p_ba1 = nc.declare_dram_parameter("ba1", [H, 1], F32, isOutput=False)
    p_w2T = nc.declare_dram_parameter("w2T", [H, H2], F32, isOutput=False)
    p_b2 = nc.declare_dram_parameter("b2", [H2, 1], F32, isOutput=False)
    p_aW2T = nc.declare_dram_parameter("aW2T", [H2, H2], F32, isOutput=False)
    p_gw2 = nc.declare_dram_parameter("gw2", [H2, H2], F32, isOutput=False)
    p_ba2 = nc.declare_dram_parameter("ba2", [H2, 1], F32, isOutput=False)
    p_wfT = nc.declare_dram_parameter("wfT", [H2, OUT], F32, isOutput=False)
    p_bfc = nc.declare_dram_parameter("bfc", [128, OUT], F32, isOutput=False)
    p_ident = nc.declare_dram_parameter("ident", [128, 128], BF16, isOutput=False)
    p_out = nc.declare_dram_parameter("out", [NSH, OUT], F32, isOutput=True)
    p_hd = nc.declare_dram_parameter("hdump", [H, NSH], F32, isOutput=True)
    K_DUMP = os.environ.get("K_DUMP", "")

    ag_ins = [nc.dram_tensor(f"ag_in{i}", [NSH, H], BF16) for i in range(4)]
    tablesA = [nc.dram_tensor(f"tableA{i}", [HALF, H], BF16, addr_space="Shared")
               for i in range(4)]
    tablesB = [nc.dram_tensor(f"tableB{i}", [HALF, H], BF16, addr_space="Shared")
               for i in range(4)]

    def wslice(s):
        lo = s * SBW * WCOL
        hi = min(NSH, (s + 1) * SBW * WCOL)
        return lo, hi - lo

    with TileContext(nc) as tc:
        with (
            tc.tile_pool(name="const", bufs=1) as cp,
            tc.tile_pool(name="xin", bufs=3) as xp,
            tc.tile_pool(name="gat", bufs=3) as gp,
            tc.tile_pool(name="stg", bufs=2) as sp,
            tc.tile_pool(name="wrk", bufs=3) as wp,
            tc.tile_pool(name="pa", bufs=2, space="PSUM") as pa,
            tc.tile_pool(name="pd", bufs=2, space="PSUM") as pd,
            tc.tile_pool(name="py", bufs=2, space="PSUM") as py,
            tc.tile_pool(name="pt", bufs=2, space="PSUM") as pt,
        ):
            # ---- persistent state + constants
            hT = cp.tile([H, NSH], F32, tag="hT")
            h2T = cp.tile([H2, NSH], F32, tag="h2T")
            s_all = cp.tile([128, TOTCH, WCOL], BF16, tag="s_all")
            t_idxA = cp.tile([128, LA // 16], I16, tag="idxA")
            t_idxB = cp.tile([128, LB // 16], I16, tag="idxB")
            w0a = cp.tile([128, H], F32, tag="w0a")
            w0b = cp.tile([128, H], F32, tag="w0b")
            b0 = cp.tile([H, 1], F32, tag="b0")
            aW1T = cp.tile([H, H], F32, tag="aW1T")
            gw1 = cp.tile([H, H], F32, tag="gw1")
            ba1 = cp.tile([H, 1], F32, tag="ba1")
            w2T = cp.tile([H, H2], F32, tag="w2T")
            b2 = cp.tile([H2, 1], F32, tag="b2")
            aW2T = cp.tile([H2, H2], F32, tag="aW2T")
            gw2 = cp.tile([H2, H2], F32, tag="gw2")
            ba2 = cp.tile([H2, 1], F32, tag="ba2")
            wfT = cp.tile([H2, OUT], F32, tag="wfT")
            bfc = cp.tile([128, OUT], F32, tag="bfc")
            ident = cp.tile([128, 128], BF16, tag="ident")

            nc.sync.dma_start(out=s_all[:, :, :], in_=p_sc[:, :, :])
            nc.sync.dma_start(out=t_idxA[:], in_=p_idxA[:, :])
            nc.sync.dma_start(out=t_idxB[:], in_=p_idxB[:, :])
            nc.sync.dma_start(out=w0a[:], in_=p_w0T[0:128, :])
            nc.sync.dma_start(out=w0b[:], in_=p_w0T[128:256, :])
            nc.sync.dma_start(out=b0[:], in_=p_b0[:, :])
            nc.sync.dma_start(out=aW1T[:], in_=p_aW1T[:, :])
            nc.sync.dma_start(out=gw1[:], in_=p_gw1[:, :])
            nc.sync.dma_start(out=ba1[:], in_=p_ba1[:, :])
            nc.sync.dma_start(out=w2T[:], in_=p_w2T[:, :])
            nc.sync.dma_start(out=b2[:], in_=p_b2[:, :])
            nc.sync.dma_start(out=aW2T[:], in_=p_aW2T[:, :])
            nc.sync.dma_start(out=gw2[:], in_=p_gw2[:, :])
            nc.sync.dma_start(out=ba2[:], in_=p_ba2[:, :])
            nc.sync.dma_start(out=wfT[:], in_=p_wfT[:, :])
            nc.sync.dma_start(out=bfc[:], in_=p_bfc[:, :])
            nc.sync.dma_start(out=ident[:], in_=p_ident[:, :])

            # ---- layer 0: hT = leaky_relu(w_hid @ x^T + b0)
            for s in range(NSB):
                lo, n = wslice(s)
                ps = pa.tile([H, 512], F32, tag="pa")
                for kc, w0t in enumerate((w0a, w0b)):
                    xt = xp.tile([128, 512], F32, tag="xt")
                    nc.sync.dma_start(out=xt[:, :n], in_=p_xT[kc * 128:(kc + 1) * 128, lo:lo + n])
                    nc.tensor.matmul(ps[:, :n], w0t[:], xt[:, :n],
                                     start=(kc == 0), stop=(kc == 1))
                t0 = wp.tile([H, 512], F32, tag="t0")
                nc.scalar.activation(t0[:, :n], ps[:, :n], AF.Identity, bias=b0[:, :])
                nc.vector.scalar_tensor_tensor(hT[:, lo:lo + n], t0[:, :n], 0.01,
                                               t0[:, :n], ALU.mult, ALU.max)

            # ---- shared conv iteration
            def stage_table(src_t, srcdim, gwt, ag_in):
                """hw^T = gwt.T @ src_t ; transpose; write node-major bf16
                rows into ag_in (cols [0:srcdim], rest pre-zeroed for H2)."""
                for s in range(NSB):
                    lo, n = wslice(s)
                    ps = pa.tile([srcdim, 512], F32, tag="pa")
                    nc.tensor.matmul(ps[:, :n], gwt[:], src_t[:, lo:lo + n],
                                     start=True, stop=True)
                    stg = sp.tile([srcdim, 512], BF16, tag="stg")
                    nc.scalar.activation(stg[:, :n], ps[:, :n], AF.Copy)
                    nt = (n + 127) // 128
                    for t in range(nt):
                        tw = min(128, n - t * 128)
                        ptt = pt.tile([128, 128], BF16, tag="pt")
                        nc.tensor.transpose(ptt[:tw, :srcdim],
                                            stg[:, t * 128:t * 128 + tw],
                                            ident[:srcdim, :srcdim])
                        rows = sp.tile([128, 128], BF16, tag="rows")
                        nc.scalar.activation(rows[:tw, :srcdim], ptt[:tw, :srcdim],
                                             AF.Copy)
                        nc.sync.dma_start(
                            out=ag_in[lo + t * 128: lo + t * 128 + tw, 0:srcdim],
                            in_=rows[:tw, :srcdim])

            K_CC = os.environ.get("K_CC", "1") == "1"
            K_GATHER = os.environ.get("K_GATHER", "1") == "1"

            def conv_iter(state_t, dim, aWt, bias_t, ag_in, tabA, tabB):
                """one antisymmetric-conv step on state_t [dim, NSH]."""
                if K_CC:
                    nc.gpsimd.collective_compute(
                        "AllGather", ALU.bypass,
                        replica_groups=[list(range(NCORES))],
                        ins=[ag_in[0:NSH // 2, :]], outs=[tabA[:, :]],
                    )
                    nc.gpsimd.collective_compute(
                        "AllGather", ALU.bypass,
                        replica_groups=[list(range(NCORES))],
                        ins=[ag_in[NSH // 2:NSH, :]], outs=[tabB[:, :]],
                    )
                offA = offB = 0
                for s in range(NSB):
                    lo, n = wslice(s)
                    ca, cb, ct = CA_sb[s], CB_sb[s], C_sb[s]
                    g = gp.tile([128, CSB_MAX, 128], BF16, tag="g")
                    GCAP = 8  # blocks per gather call (<=1024 idx: SWDGE ring)
                    if K_GATHER:
                        qn = 0
                        for b0 in range(0, ca, GCAP):
                            b1 = min(ca, b0 + GCAP)
                            o = offA + b0 * CHUNK
                            nc.gpsimd.dma_gather(
                                out_ap=g[:, b0:b1, :], in_ap=tabA[:, :],
                                idxs_ap=t_idxA[:, o // 16:(o + (b1 - b0) * CHUNK) // 16],
                                num_idxs=(b1 - b0) * CHUNK,
                                num_idxs_reg=(b1 - b0) * CHUNK,
                                elem_size=H, queue_num=qn % 4)
                            qn += 1
                        for b0 in range(0, cb, GCAP):
                            b1 = min(cb, b0 + GCAP)
                            o = offB + b0 * CHUNK
                            nc.gpsimd.dma_gather(
                                out_ap=g[:, ca + b0:ca + b1, :], in_ap=tabB[:, :],
                                idxs_ap=t_idxB[:, o // 16:(o + (b1 - b0) * CHUNK) // 16],
                                num_idxs=(b1 - b0) * CHUNK,
                                num_idxs_reg=(b1 - b0) * CHUNK,
                                elem_size=H, queue_num=qn % 4)
                            qn += 1
                    offA += ca * CHUNK
                    offB += cb * CHUNK

                    psy = py.tile([dim, 512], F32, tag="py")
                    first = True   # start=True resets has_written for the
                    for w in sb_windows[s]:   # whole bank: exactly one per sb
                        colb = (w % SBW) * WCOL
                        for k in range(int(CA[w])):
                            blk = blkA_off[s][w] + k
                            nc.tensor.matmul(
                                psy[:, colb:colb + WCOL],
                                g[:, blk, 0:dim],
                                s_all[:, int(choff[s]) + blk, :],
                                start=first, stop=False,
                                skip_group_check=True)
                            first = False
                        for k in range(int(CB[w])):
                            blk = ca + blkB_off[s][w] + k
                            nc.tensor.matmul(
                                psy[:, colb:colb + WCOL],
                                g[:, blk, 0:dim],
                                s_all[:, int(choff[s]) + blk, :],
                                start=first, stop=False,
                                skip_group_check=True)
                            first = False

                    nc.tensor.matmul(psy[:, :n], aWt[:], state_t[:, lo:lo + n],
                                     start=False, stop=True, skip_group_check=True)
                    upd = wp.tile([dim, 512], F32, tag="upd")
                    nc.scalar.activation(upd[:, :n], psy[:, :n], AF.Tanh, bias=bias_t[:, :])
                    nc.vector.scalar_tensor_tensor(
                        state_t[:, lo:lo + n], upd[:, :n], EPS,
                        state_t[:, lo:lo + n], ALU.mult, ALU.add)

            if K_DUMP == "h0":
                nc.sync.dma_start(out=p_hd[:, :], in_=hT[:, :])
            # ---- conv1 x3
            for it in range(int(os.environ.get("K_IT1", "3"))):
                stage_table(hT, H, gw1, ag_ins[it])
                conv_iter(hT, H, aW1T, ba1, ag_ins[it], tablesA[it], tablesB[it])
                if K_DUMP == f"it{it+1}":
                    nc.sync.dma_start(out=p_hd[:, :], in_=hT[:, :])

            # ---- transition: g = lrelu(hT); h2T = lrelu(w_hid2 @ g + b2)
            for s in range(NSB):
                lo, n = wslice(s)
                gk = wp.tile([H, 512], F32, tag="tsum")
                nc.vector.scalar_tensor_tensor(gk[:, :n], hT[:, lo:lo + n], 0.01,
                                               hT[:, lo:lo + n], ALU.mult, ALU.max)
                ps = pa.tile([H2, 512], F32, tag="pa")
                nc.tensor.matmul(ps[:, :n], w2T[:], gk[:, :n], start=True, stop=True)
                t2 = wp.tile([H2, 512], F32, tag="upd")
                nc.scalar.activation(t2[:, :n], ps[:, :n], AF.Identity, bias=b2[:, :])
                nc.vector.scalar_tensor_tensor(h2T[:, lo:lo + n], t2[:, :n], 0.01,
                                               t2[:, :n], ALU.mult, ALU.max)

            # ---- conv2 x1 (table rows are [hw2 | junk]; junk cols unread)
            if os.environ.get("K_IT2", "1") == "1":
                stage_table(h2T, H2, gw2, ag_ins[3])
                conv_iter(h2T, H2, aW2T, ba2, ag_ins[3], tablesA[3], tablesB[3])
            if K_DUMP == "h2":
                nc.sync.dma_start(out=p_hd[:64, :], in_=h2T[:, :])

            # ---- final: logits + log_softmax, node-major
            NT = (NSH + 127) // 128
            for t in range(NT):
                tw = min(128, NSH - t * 128)
                pf = pd.tile([128, OUT], F32, tag="pd")
                nc.tensor.matmul(pf[:tw, :], h2T[:, t * 128:t * 128 + tw],
                                 wfT[:], start=True, stop=True)
                lg = wp.tile([128, OUT], F32, tag="lg")
                nc.vector.tensor_tensor(lg[:tw, :], pf[:tw, :], bfc[:tw, :], ALU.add)
                nmx = wp.tile([128, 1], F32, tag="nmx")
                nc.vector.tensor_reduce(nmx[:tw, :], lg[:tw, :],
                                        mybir.AxisListType.X, ALU.max, negate=True)
                ex = wp.tile([128, OUT], F32, tag="ex")
                se = wp.tile([128, 1], F32, tag="se")
                nc.scalar.activation(ex[:tw, :], lg[:tw, :], AF.Exp,
                                     bias=nmx[:tw, :], accum_out=se[:tw, :])
                lse = wp.tile([128, 1], F32, tag="lse")
                nc.scalar.activation(lse[:tw, :], se[:tw, :], AF.Ln)
                shift = wp.tile([128, 1], F32, tag="shift")
                nc.vector.tensor_tensor(shift[:tw, :], nmx[:tw, :], lse[:tw, :], ALU.subtract)
                ot = wp.tile([128, OUT], F32, tag="ot")
                nc.vector.tensor_scalar(ot[:tw, :], lg[:tw, :], shift[:tw, :],
                                        None, ALU.add)
                nc.sync.dma_start(out=p_out[t * 128:t * 128 + tw, :], in_=ot[:tw, :])

    nc.finalize()
    return nc


# ----------------------------------------------------------------- run
_CACHE = {}


def kernel(x, edge_index, w_hid, b_hid, W_a1, gcn_w1, b_a1,
           w_hid2, b_hid2, W_a2, gcn_w2, b_a2, w_fc, b_fc, _trace=False):
    x = np.asarray(x, np.float32)
    meta, percore = _prep_graph(edge_index)
    nc = _build(meta)

    f32 = np.float32
    w0T = np.ascontiguousarray(np.asarray(w_hid, f32).T)            # [256,128]
    aW1 = np.asarray(W_a1, f32)
    aW1T = np.ascontiguousarray(aW1.T - aW1 - GAMMA * np.eye(H, dtype=f32))
    aW2 = np.asarray(W_a2, f32)
    aW2T = np.ascontiguousarray(aW2.T - aW2 - GAMMA * np.eye(H2, dtype=f32))
    common = dict(
        w0T=w0T,
        b0=np.asarray(b_hid, f32).reshape(H, 1),
        aW1T=aW1T,
        gw1=np.ascontiguousarray(np.asarray(gcn_w1, f32)),
        ba1=np.asarray(b_a1, f32).reshape(H, 1),
        w2T=np.ascontiguousarray(np.asarray(w_hid2, f32).T),
        b2=np.asarray(b_hid2, f32).reshape(H2, 1),
        aW2T=aW2T,
        gw2=np.ascontiguousarray(np.asarray(gcn_w2, f32)),
        ba2=np.asarray(b_a2, f32).reshape(H2, 1),
        wfT=np.ascontiguousarray(np.asarray(w_fc, f32).T),
        bfc=np.tile(np.asarray(b_fc, f32).reshape(1, OUT), (128, 1)),
        ident=np.eye(128, dtype=mybir.dt.np(BF16)),
    )
    in_maps = []
    for c in range(NCORES):
        xT = np.ascontiguousarray(x[c * NSH:(c + 1) * NSH].T)
        in_maps.append({"xT": xT, **percore[c], **common})

    res = run_bass_kernel_spmd(nc, in_maps, list(range(NCORES)), trace=_trace)
    out = np.concatenate([res.results[c]["out"] for c in range(NCORES)], axis=0)
    kernel.last_hdump = np.stack([res.results[c]["hdump"] for c in range(NCORES)])
    kernel.last_exec_time_ns = res.exec_time_ns
    kernel.last_results = res
    return out
